# revision 9
# baseline (speedup 1.0000x reference)
"""Distributed brute-force kNN retrieval (cosine similarity) on 8 Trainium2 cores.

Strategy (per spec sharding hint, adapted):
  - Shard the feature bank along N across 8 cores (62500 rows each).
  - Host pre-transposes each shard to [768, 62500] (fp8 e4m3) so the device
    can DMA contraction-major tiles directly (no on-device transpose needed).
  - Each core computes raw dot products q @ f_shard.T with fp8 matmuls
    (fp32 PSUM accumulation). Feature chunks are processed two-at-a-time via
    PE column tiling (tile_position=(0,64)): chunk A lands on PSUM partitions
    0-63, chunk B on 64-127, so all 128 partitions carry similarities.
  - The DVE Max8/MaxIndex instructions extract the top-8 candidates per
    query-row per similarity block. Blocks shrink toward the end of the
    stream so the final Max8 calls (the serial tail after the last DMA) are
    cheap. The odd 125th chunk is covered by an overlapping final pair
    (chunks 123,124); the host dedups.
  - Host maps candidates to global row indices, rescores them exactly in fp32
    (normalized cosine similarity, same math as the reference), does the final
    top-k reduction, and gathers the data segments.

The candidate margin (top-8 of every <=4500-col half-block when only the
global top-5 is needed) makes the device pass insensitive to fp8 rounding: a
true top-5 entry would have to be pushed below rank 8 *within its own block*
by fp8 dot-noise (sigma ~0.7) across gaps that total many sigma. The exact
host rescore then removes all remaining matmul error.
"""

import numpy as np

import concourse.bacc as bacc
import concourse.mybir as mybir
from concourse.tile import TileContext
from concourse.bass_utils import run_bass_kernel_spmd

# Problem geometry (hardcoded per spec).
B = 64             # queries
D = 768            # feature dim
N = 500000         # feature rows
NCORES = 8
NSH = N // NCORES  # 62500 rows per core
KC = D // 128      # 6 contraction chunks of 128
CHUNK = 500        # matmul moving free dim (one PSUM bank)
NCHUNKS = NSH // CHUNK           # 125 chunks
NPAIRS = 63                      # pairs (2j, 2j+1); pair 62 = (123, 124) overlaps
BLOCKS = [9, 9, 9, 9, 9, 9, 5, 3, 1]   # pairs per Max8 block (sum = 63)
NBLOCKS = len(BLOCKS)
BLOCK_BASE = [sum(BLOCKS[:i]) for i in range(NBLOCKS)]
GRP = 8                          # chunks per DMA group (3 MB fp8)
TOPB = 8                         # Max8 output width per block
MAXBP = max(BLOCKS)

_COMPILED = None
LAST_RESULTS = None  # test harness introspection


def _pair_chunks(j):
    return (2 * j, 2 * j + 1) if j < 62 else (123, 124)


def _build():
    nc = bacc.Bacc("TRN2", target_bir_lowering=False, debug=False)
    qT = nc.declare_dram_parameter("qT", [D, B], mybir.dt.float8e4, isOutput=False)
    fT = nc.declare_dram_parameter("fT", [D, NSH], mybir.dt.float8e4, isOutput=False)
    out_vals = nc.declare_dram_parameter(
        "vals", [128, NBLOCKS * TOPB], mybir.dt.float32, isOutput=True
    )
    out_idx = nc.declare_dram_parameter(
        "idx", [128, NBLOCKS * TOPB], mybir.dt.uint32, isOutput=True
    )

    qT_r = qT.ap().rearrange("(k p) m -> p k m", p=128)
    fT_r = fT.ap().rearrange("(k p) n -> p k n", p=128)

    with TileContext(nc) as tc:
        with (
            tc.tile_pool(name="qpool", bufs=1) as qpool,
            tc.tile_pool(name="fpool", bufs=4) as fpool,
            tc.tile_pool(name="simspool", bufs=3) as simspool,
            tc.tile_pool(name="outpool", bufs=1) as outpool,
            tc.tile_pool(name="psum", bufs=6, space="PSUM") as psump,
        ):
            q_sb = qpool.tile([128, KC, B], mybir.dt.float8e4)
            nc.sync.dma_start(out=q_sb[:], in_=qT_r)

            vals_st = outpool.tile([128, NBLOCKS * TOPB], mybir.dt.float32)
            idx_st = outpool.tile([128, NBLOCKS * TOPB], mybir.dt.uint32)

            f_tiles = {}
            loaded = [0]  # chunks loaded so far

            def load_until(c):
                while loaded[0] <= c:
                    gc0 = loaded[0]
                    gchunks = min(GRP, NCHUNKS - gc0)
                    f_sb = fpool.tile([128, KC, GRP * CHUNK], mybir.dt.float8e4)
                    nc.sync.dma_start(
                        out=f_sb[:, :, :gchunks * CHUNK],
                        in_=fT_r[:, :, gc0 * CHUNK:(gc0 + gchunks) * CHUNK],
                    )
                    for c2 in range(gchunks):
                        f_tiles[gc0 + c2] = f_sb[:, :, c2 * CHUNK:(c2 + 1) * CHUNK]
                    loaded[0] = gc0 + gchunks

            def mm_half(ps, chunk, half):
                for k in range(KC):
                    nc.tensor.matmul(
                        ps[half * B:(half + 1) * B, :],
                        lhsT=q_sb[:, k, :],
                        rhs=f_tiles[chunk][:, k, :],
                        start=(k == 0),
                        stop=(k == KC - 1),
                        tile_position=(0, half * B) if half else None,
                    )

            for blk in range(NBLOCKS):
                bpairs = BLOCKS[blk]
                bsize = bpairs * CHUNK
                sims = simspool.tile([128, MAXBP * CHUNK], mybir.dt.float32)
                for j in range(bpairs):
                    ca, cb = _pair_chunks(BLOCK_BASE[blk] + j)
                    load_until(cb)
                    ps = psump.tile([128, CHUNK], mybir.dt.float32)
                    mm_half(ps, ca, 0)
                    mm_half(ps, cb, 1)
                    nc.scalar.copy(
                        out=sims[:, j * CHUNK:(j + 1) * CHUNK], in_=ps[:]
                    )
                nc.vector.max(
                    out=vals_st[:, blk * TOPB:(blk + 1) * TOPB],
                    in_=sims[:, :bsize],
                )
                nc.vector.max_index(
                    out=idx_st[:, blk * TOPB:(blk + 1) * TOPB],
                    in_max=vals_st[:, blk * TOPB:(blk + 1) * TOPB],
                    in_values=sims[:, :bsize],
                )

            nc.sync.dma_start(out=out_vals.ap(), in_=vals_st[:])
            nc.sync.dma_start(out=out_idx.ap(), in_=idx_st[:])

    nc.compile()
    return nc


def _get_compiled():
    global _COMPILED
    if _COMPILED is None:
        _COMPILED = _build()
    return _COMPILED


def _candidate_indices(idx_arr):
    """Map device Max8 indices (128, NBLOCKS*8) to shard-local feature rows.

    Row p < 64 is query p over the first chunk of each pair; row p >= 64 is
    query p-64 over the second chunk. Block b covers pairs starting at
    BLOCK_BASE[b]; a Max8 index i within the block means pair
    BLOCK_BASE[b] + i//CHUNK at position i%CHUNK.
    """
    out = []
    for q in range(B):
        rows = []
        for half in (0, 1):
            v = idx_arr[q + half * B].astype(np.int64)  # (NBLOCKS*TOPB,)
            for blk in range(NBLOCKS):
                i = v[blk * TOPB:(blk + 1) * TOPB]
                pair = BLOCK_BASE[blk] + i // CHUNK
                chunk = np.where(pair < 62, 2 * pair + half, 123 + half)
                rows.append(chunk * CHUNK + i % CHUNK)
        out.append(np.concatenate(rows))
    return np.stack(out)  # (B, 2*NBLOCKS*TOPB)


def kernel(query_feature, feature, data, k=5, **kwargs):
    global LAST_RESULTS
    q = np.ascontiguousarray(np.asarray(query_feature, dtype=np.float32))
    f = np.asarray(feature, dtype=np.float32)
    data = np.asarray(data)
    k = int(k)
    assert q.shape == (B, D) and f.shape == (N, D)

    nc = _get_compiled()

    F8 = mybir.dt.np(mybir.dt.float8e4)
    qT = np.ascontiguousarray(q.T.astype(F8))
    in_maps = []
    for i in range(NCORES):
        fT = np.ascontiguousarray(f[i * NSH:(i + 1) * NSH].T.astype(F8))
        in_maps.append({"qT": qT, "fT": fT})

    res = run_bass_kernel_spmd(nc, in_maps, core_ids=list(range(NCORES)))
    LAST_RESULTS = res

    cand = np.concatenate(
        [
            i * NSH + _candidate_indices(res.results[i]["idx"])
            for i in range(NCORES)
        ],
        axis=1,
    )  # (B, NCORES * 2*NBLOCKS*TOPB)

    # Exact fp32 rescore of candidates (same math as the reference).
    qn = q / np.linalg.norm(q, axis=1, keepdims=True)
    fc = f[cand]  # (B, C, D)
    fn = fc / np.linalg.norm(fc, axis=2, keepdims=True)
    sims = np.einsum("bd,bcd->bc", qn, fn)  # fp32

    # Final top-k with jax.lax.top_k tie-breaking (value desc, index asc).
    # Chunk 123/124 features can appear twice (overlapping final pair):
    # sort by index, mask duplicate neighbors.
    o = np.argsort(cand, axis=1, kind="stable")
    cand_s = np.take_along_axis(cand, o, axis=1)
    sims_s = np.take_along_axis(sims, o, axis=1)
    dup = np.zeros_like(sims_s, dtype=bool)
    dup[:, 1:] = cand_s[:, 1:] == cand_s[:, :-1]
    sims_s = np.where(dup, -np.inf, sims_s)
    sel = np.argsort(-sims_s, axis=1, kind="stable")[:, :k]
    top_idx = np.take_along_axis(cand_s, sel, axis=1)  # (B, k)

    return data[top_idx]  # (B, k, data_cols), input dtype preserved


# revision 11
# speedup vs baseline: 1.1076x; 1.1076x over previous
"""Distributed brute-force kNN retrieval (cosine similarity) on 8 Trainium2 cores.

Strategy (per spec sharding hint, adapted):
  - Shard the feature bank along N across 8 cores (62500 rows each).
  - Host pre-transposes each shard to [768, 62500] (fp8 e4m3) so the device
    can DMA contraction-major tiles directly (no on-device transpose needed).
  - Each core computes raw dot products q @ f_shard.T with fp8 matmuls
    (fp32 PSUM accumulation). Feature chunks are processed two-at-a-time via
    PE column tiling (tile_position=(0,64)): chunk A lands on PSUM partitions
    0-63, chunk B on 64-127, so all 128 partitions carry similarities.
  - The DVE Max8/MaxIndex instructions extract the top-8 candidates per
    query-row per similarity block. Blocks shrink toward the end of the
    stream so the final Max8 calls (the serial tail after the last DMA) are
    cheap. The odd 125th chunk is covered by an overlapping final pair
    (chunks 123,124); the host dedups.
  - Host maps candidates to global row indices, rescores them exactly in fp32
    (normalized cosine similarity, same math as the reference), does the final
    top-k reduction, and gathers the data segments.

The candidate margin (top-8 of every <=4500-col half-block when only the
global top-5 is needed) makes the device pass insensitive to fp8 rounding: a
true top-5 entry would have to be pushed below rank 8 *within its own block*
by fp8 dot-noise (sigma ~0.7) across gaps that total many sigma. The exact
host rescore then removes all remaining matmul error.
"""

import os
import sys

import numpy as np

import concourse.bacc as bacc
import concourse.mybir as mybir
from concourse.tile import TileContext
from concourse.bass_utils import run_bass_kernel_spmd


def _ensure_ntff_hook():
    """run_bass_kernel_spmd(trace) under axon imports antenv.axon_hooks,
    which this container image lacks. Provide the shim (profiling works) or
    disable tracing so a stray BASS_TRACE env var cannot crash the run."""
    try:
        import antenv.axon_hooks  # noqa: F401
        return
    except ImportError:
        pass
    try:
        import types
        from trn_agent_boot.trn_boot import _ntff_profile_via_ctypes
        hook = _ntff_profile_via_ctypes("/opt/axon/libaxon_pjrt.so")
        mod = types.ModuleType("antenv.axon_hooks")
        mod.get_axon_ntff_profile_hook = lambda: hook
        mod.set_axon_ntff_profile_hook = lambda h: None
        sys.modules["antenv.axon_hooks"] = mod
        import antenv
        antenv.axon_hooks = mod
    except Exception:
        os.environ["BASS_NEVER_TRACE"] = "1"

# Problem geometry (hardcoded per spec).
B = 64             # queries
D = 768            # feature dim
N = 500000         # feature rows
NCORES = 8
NSH = N // NCORES  # 62500 rows per core
KC = D // 128      # 6 contraction chunks of 128
CHUNK = 500        # matmul moving free dim (one PSUM bank)
NCHUNKS = NSH // CHUNK           # 125 chunks
NPAIRS = 63                      # pairs (2j, 2j+1); pair 62 = (123, 124) overlaps
BLOCKS = [9, 9, 9, 9, 9, 9, 5, 3, 1]   # pairs per Max8 block (sum = 63)
NBLOCKS = len(BLOCKS)
BLOCK_BASE = [sum(BLOCKS[:i]) for i in range(NBLOCKS)]
GRP = 8                          # chunks per DMA group (3 MB fp8)
TOPB = 8                         # Max8 output width per block
MAXBP = max(BLOCKS)

_COMPILED = None
LAST_RESULTS = None  # test harness introspection


def _pair_chunks(j):
    return (2 * j, 2 * j + 1) if j < 62 else (123, 124)


def _build():
    nc = bacc.Bacc("TRN2", target_bir_lowering=False, debug=False)
    qT = nc.declare_dram_parameter("qT", [D, B], mybir.dt.float8e4, isOutput=False)
    fT = nc.declare_dram_parameter("fT", [D, NSH], mybir.dt.float8e4, isOutput=False)
    out_vals = nc.declare_dram_parameter(
        "vals", [128, NBLOCKS * TOPB], mybir.dt.float32, isOutput=True
    )
    out_idx = nc.declare_dram_parameter(
        "idx", [128, NBLOCKS * TOPB], mybir.dt.uint32, isOutput=True
    )

    qT_r = qT.ap().rearrange("(k p) m -> p k m", p=128)
    fT_r = fT.ap().rearrange("(k p) n -> p k n", p=128)

    with TileContext(nc) as tc:
        with (
            tc.tile_pool(name="qpool", bufs=1) as qpool,
            tc.tile_pool(name="fpool", bufs=4) as fpool,
            tc.tile_pool(name="simspool", bufs=3) as simspool,
            tc.tile_pool(name="outpool", bufs=1) as outpool,
            tc.tile_pool(name="psum", bufs=6, space="PSUM") as psump,
        ):
            q_sb = qpool.tile([128, KC, B], mybir.dt.float8e4)
            nc.sync.dma_start(out=q_sb[:], in_=qT_r)

            vals_st = outpool.tile([128, NBLOCKS * TOPB], mybir.dt.float32)
            idx_st = outpool.tile([128, NBLOCKS * TOPB], mybir.dt.uint32)

            f_tiles = {}
            loaded = [0]  # chunks loaded so far

            def load_until(c):
                while loaded[0] <= c:
                    gc0 = loaded[0]
                    gchunks = min(GRP, NCHUNKS - gc0)
                    f_sb = fpool.tile([128, KC, GRP * CHUNK], mybir.dt.float8e4)
                    nc.sync.dma_start(
                        out=f_sb[:, :, :gchunks * CHUNK],
                        in_=fT_r[:, :, gc0 * CHUNK:(gc0 + gchunks) * CHUNK],
                    )
                    for c2 in range(gchunks):
                        f_tiles[gc0 + c2] = f_sb[:, :, c2 * CHUNK:(c2 + 1) * CHUNK]
                    loaded[0] = gc0 + gchunks

            def mm_half(ps, chunk, half):
                for k in range(KC):
                    nc.tensor.matmul(
                        ps[half * B:(half + 1) * B, :],
                        lhsT=q_sb[:, k, :],
                        rhs=f_tiles[chunk][:, k, :],
                        start=(k == 0),
                        stop=(k == KC - 1),
                        tile_position=(0, half * B) if half else None,
                    )

            for blk in range(NBLOCKS):
                bpairs = BLOCKS[blk]
                bsize = bpairs * CHUNK
                sims = simspool.tile([128, MAXBP * CHUNK], mybir.dt.float32)
                for j in range(bpairs):
                    ca, cb = _pair_chunks(BLOCK_BASE[blk] + j)
                    load_until(cb)
                    ps = psump.tile([128, CHUNK], mybir.dt.float32)
                    mm_half(ps, ca, 0)
                    mm_half(ps, cb, 1)
                    nc.scalar.copy(
                        out=sims[:, j * CHUNK:(j + 1) * CHUNK], in_=ps[:]
                    )
                nc.vector.max(
                    out=vals_st[:, blk * TOPB:(blk + 1) * TOPB],
                    in_=sims[:, :bsize],
                )
                nc.vector.max_index(
                    out=idx_st[:, blk * TOPB:(blk + 1) * TOPB],
                    in_max=vals_st[:, blk * TOPB:(blk + 1) * TOPB],
                    in_values=sims[:, :bsize],
                )

            nc.sync.dma_start(out=out_vals.ap(), in_=vals_st[:])
            nc.sync.dma_start(out=out_idx.ap(), in_=idx_st[:])

    nc.compile()
    return nc


def _get_compiled():
    global _COMPILED
    if _COMPILED is None:
        _COMPILED = _build()
    return _COMPILED


def _candidate_indices(idx_arr):
    """Map device Max8 indices (128, NBLOCKS*8) to shard-local feature rows.

    Row p < 64 is query p over the first chunk of each pair; row p >= 64 is
    query p-64 over the second chunk. Block b covers pairs starting at
    BLOCK_BASE[b]; a Max8 index i within the block means pair
    BLOCK_BASE[b] + i//CHUNK at position i%CHUNK.
    """
    out = []
    for q in range(B):
        rows = []
        for half in (0, 1):
            v = idx_arr[q + half * B].astype(np.int64)  # (NBLOCKS*TOPB,)
            for blk in range(NBLOCKS):
                i = v[blk * TOPB:(blk + 1) * TOPB]
                pair = BLOCK_BASE[blk] + i // CHUNK
                chunk = np.where(pair < 62, 2 * pair + half, 123 + half)
                rows.append(chunk * CHUNK + i % CHUNK)
        out.append(np.concatenate(rows))
    return np.stack(out)  # (B, 2*NBLOCKS*TOPB)


def kernel(query_feature, feature, data, k=5, **kwargs):
    global LAST_RESULTS
    q = np.ascontiguousarray(np.asarray(query_feature, dtype=np.float32))
    f = np.asarray(feature, dtype=np.float32)
    data = np.asarray(data)
    k = int(k)
    assert q.shape == (B, D) and f.shape == (N, D)

    nc = _get_compiled()

    F8 = mybir.dt.np(mybir.dt.float8e4)
    qT = np.ascontiguousarray(q.T.astype(F8))
    in_maps = []
    for i in range(NCORES):
        fT = np.ascontiguousarray(f[i * NSH:(i + 1) * NSH].T.astype(F8))
        in_maps.append({"qT": qT, "fT": fT})

    _ensure_ntff_hook()
    res = run_bass_kernel_spmd(nc, in_maps, core_ids=list(range(NCORES)))
    LAST_RESULTS = res

    cand = np.concatenate(
        [
            i * NSH + _candidate_indices(res.results[i]["idx"])
            for i in range(NCORES)
        ],
        axis=1,
    )  # (B, NCORES * 2*NBLOCKS*TOPB)

    # Exact fp32 rescore of candidates (same math as the reference).
    qn = q / np.linalg.norm(q, axis=1, keepdims=True)
    fc = f[cand]  # (B, C, D)
    fn = fc / np.linalg.norm(fc, axis=2, keepdims=True)
    sims = np.einsum("bd,bcd->bc", qn, fn)  # fp32

    # Final top-k with jax.lax.top_k tie-breaking (value desc, index asc).
    # Chunk 123/124 features can appear twice (overlapping final pair):
    # sort by index, mask duplicate neighbors.
    o = np.argsort(cand, axis=1, kind="stable")
    cand_s = np.take_along_axis(cand, o, axis=1)
    sims_s = np.take_along_axis(sims, o, axis=1)
    dup = np.zeros_like(sims_s, dtype=bool)
    dup[:, 1:] = cand_s[:, 1:] == cand_s[:, :-1]
    sims_s = np.where(dup, -np.inf, sims_s)
    sel = np.argsort(-sims_s, axis=1, kind="stable")[:, :k]
    top_idx = np.take_along_axis(cand_s, sel, axis=1)  # (B, k)

    return data[top_idx]  # (B, k, data_cols), input dtype preserved
